# revision 44
# baseline (speedup 1.0000x reference)
"""Sliding-window MQA attention block on 8 Trainium2 NeuronCores.

Sharding: sequence-parallel. 8 cores = 2 batches x 4 query-chunks of 512
tokens. Each core loads its 512 query tokens plus a 256-token K/V halo
(768 KV tokens total, zero-padded in front for chunk 0), computes the
Q/K/V projections, windowed attention for all 16 heads, and the final
projection locally. No collectives; the host concatenates chunk outputs
and adds the output bias.

Device algorithm (per core). Logits are computed directly TRANSPOSED
([s, t]: key position on partitions, query position free) so the
probs @ V contraction needs no PE transposes at all:
  ktd[128, 768]  = K^T computed on partitions 0-63, duplicated to the
                   upper half by an SBUF->SBUF DMA (MQA shared K)
  vaug[sb][128, 65] = V s-block with an all-ones column (softmax denom)
  qT[1024, 512]  = WqT.T @ xqT   (per 128-row block mb, interleaved with
                                  the attention of heads 2mb, 2mb+1)
  per head h:
    logitsT packed into pl[128, 3, 512] (3 PSUM banks, exactly filled):
      seven matmuls  pl[:, bank, off:off+w] = ktd_sb.T @ qh[:, t0:t0+w]
    probsT = exp(0.125 * pl) * band     (one ACT op + one DVE mul)
    po[t, 65] = sum_sb probsT_sb.T @ vaug[sb]   (PSUM accumulation)
    attn[t, 64h:64h+64] = po[:, :64] * (1 / po[:, 64])
  attnT via hardware DMA (xbar) transpose off the compute engines;
  final[512, 1024] = attnT.T @ WfT (the bias is added on the host).

Logits tiles, Q-proj tiles and all small PSUM tiles come from 1-bank
pools run several slots deep so consecutive heads pipeline across
PE -> ACT(exp) -> DVE(mask) -> PE(probs@V). Inputs load as one large
strided DMA per tensor, split across the two HWDGE rings (sync/scalar)
plus the gpsimd SWDGE ring so the rings drain in parallel.
"""

import os
import sys

import numpy as np

for _p in ("/opt/trn_rl_repo",):
    if _p not in sys.path and os.path.isdir(_p):
        sys.path.insert(0, _p)

import ml_dtypes

import concourse.bass as bass
import concourse.mybir as mybir
import concourse.tile as tile
from concourse import bacc
from concourse.bass_utils import run_bass_kernel_spmd

WIDTH = 1024
H = 16
HD = 64
WIN = 256
T = 512          # query tokens per core
KV = 768         # kv tokens per core (256 halo + 512)
NKB = WIDTH // 128
NTB = T // 128
NSB = KV // 128
F32 = mybir.dt.float32

USE_BF16 = os.environ.get("KERNEL_F32", "0") != "1"
DT = mybir.dt.bfloat16 if USE_BF16 else mybir.dt.float32
NPDT = ml_dtypes.bfloat16 if USE_BF16 else np.float32

# Packed [s, t] logits layout: (bank, col_off, width, sb, t0).
# Tile (bank, off..off+w) holds logitsT[s in 128*sb block, t in t0..t0+w].
# Each matmul stays inside one 512-f32 PSUM bank; 3 banks exactly filled.
# Each bank is its own 1-bank PSUM tile from a 6-slot pool, so the
# PE->exp->mask->PV chain pipelines ~2 heads deep per bank.
SEGS = (
    (0, 0, 128, 0, 0),
    (0, 128, 256, 1, 0),
    (0, 384, 128, 5, 384),
    (1, 0, 384, 2, 0),
    (1, 384, 128, 4, 256),
    (2, 0, 384, 3, 128),
    (2, 384, 128, 4, 384),
)

# probs @ V source map: for each query block tb, the three contributing
# key blocks sb and where their [s, 128t] slice lives in the packed tile.
PV = (
    ((0, 0, 0), (1, 0, 128), (2, 1, 0)),
    ((1, 0, 256), (2, 1, 128), (3, 2, 0)),
    ((2, 1, 256), (3, 2, 128), (4, 1, 384)),
    ((3, 2, 256), (4, 2, 384), (5, 0, 384)),
)


def build_kernel(reps=1):
    """reps > 1 loops the whole body (loads + compute) on-device with
    tc.For_i — used by test.py to measure marginal per-iteration HW time."""
    nc = bacc.Bacc(None, target_bir_lowering=False)

    xkvT_d = nc.dram_tensor("xkvT", [WIDTH, KV], DT, kind="ExternalInput")
    wqT_d = nc.dram_tensor("wqT", [WIDTH, WIDTH], DT, kind="ExternalInput")
    wkT_d = nc.dram_tensor("wkT", [WIDTH, HD], DT, kind="ExternalInput")
    wvT_d = nc.dram_tensor("wvT", [WIDTH, HD], DT, kind="ExternalInput")
    wfT_d = nc.dram_tensor("wfT", [WIDTH, WIDTH], DT, kind="ExternalInput")
    band_d = nc.dram_tensor("band", [128, 3, 512], DT, kind="ExternalInput")
    out_d = nc.dram_tensor("out", [T, WIDTH], F32, kind="ExternalOutput")

    with tile.TileContext(nc) as tc:
        from contextlib import nullcontext

        loop = (
            tc.For_i(
                0,
                reps,
                1,
                hint_engines=(
                    mybir.EngineType.PE,
                    mybir.EngineType.Activation,
                    mybir.EngineType.DVE,
                    mybir.EngineType.SP,
                    mybir.EngineType.Pool,
                ),
                staggered_reset=True,
            )
            if reps > 1
            else nullcontext()
        )
        with tc.tile_pool(name="persist", bufs=1) as pp, loop:
            # ---- load inputs: one strided DMA per tensor. Two HWDGE rings
            # run in parallel: small tensors on the scalar ring, the big
            # ones on the sync ring (each ring drains FIFO).
            wk_all = pp.tile([128, NKB, HD], DT, tag="wk")
            nc.scalar.dma_start(
                wk_all[:], wkT_d[:, :].rearrange("(a p) j -> p a j", p=128)
            )
            xkv_all = pp.tile([128, NKB, KV], DT, tag="xkv")
            xkv_dram = xkvT_d[:, :].rearrange("(a p) j -> p a j", p=128)
            nc.sync.dma_start(xkv_all[:, 0:4, :], xkv_dram[:, 0:4, :])
            nc.scalar.dma_start(xkv_all[:, 4:8, :], xkv_dram[:, 4:8, :])
            wq_all = pp.tile([128, NKB, WIDTH], DT, tag="wq")
            wq_dram = wqT_d[:, :].rearrange("(a p) j -> p a j", p=128)
            nc.sync.dma_start(wq_all[:, :, 0:512], wq_dram[:, :, 0:512])
            nc.sync.dma_start(wq_all[:, :, 512:1024], wq_dram[:, :, 512:1024])
            wv_all = pp.tile([128, NKB, HD], DT, tag="wv")
            nc.gpsimd.dma_start(
                wv_all[:], wvT_d[:, :].rearrange("(a p) j -> p a j", p=128)
            )
            band_t = pp.tile([128, 3, 512], DT, tag="band")
            nc.gpsimd.dma_start(band_t[:], band_d[:, :, :])
            wf_all = pp.tile([128, NKB, WIDTH], DT, tag="wf")
            nc.sync.dma_start(
                wf_all[:], wfT_d[:, :].rearrange("(a p) j -> p a j", p=128)
            )

            # ---- persistent intermediates ----
            qT_t = [pp.tile([128, T], DT, tag=f"qT{i}", name=f"qT{i}") for i in range(NKB)]
            ktd = pp.tile([128, KV], DT, tag="ktd")
            vaug = [pp.tile([128, HD + 1], DT, tag=f"vaug{i}", name=f"vaug{i}") for i in range(NSB)]
            attn_t = [pp.tile([128, WIDTH], DT, tag=f"attn{i}", name=f"attn{i}") for i in range(NTB)]
            attnT_t = [pp.tile([128, T], DT, tag=f"attnT{i}", name=f"attnT{i}") for i in range(NKB)]

            with (
                tc.tile_pool(name="psplb", bufs=5, space="PSUM") as psplb,
                tc.tile_pool(name="pssm", bufs=3, space="PSUM") as pssm,
                tc.tile_pool(name="awork", bufs=4) as awork,
            ):
                # ---- K/V projections (needed by every head; do first) ----
                # K^T computed once on partitions 0-63, then duplicated to
                # partitions 64-127 with an SBUF->SBUF DMA (MQA shared K).
                pk0 = psplb.tile([128, 512], F32, tag="plb", name="pk0")
                pk1 = psplb.tile([128, 512], F32, tag="plb", name="pk1")
                for seg0, segw, pk in ((0, 512, pk0), (512, 256, pk1)):
                    for kb in range(NKB):
                        nc.tensor.matmul(
                            pk[0:HD, 0:segw],
                            lhsT=wk_all[:, kb, :],
                            rhs=xkv_all[:, kb, seg0 : seg0 + segw],
                            start=(kb == 0),
                            stop=(kb == NKB - 1),
                        )
                nc.vector.tensor_copy(ktd[0:HD, 0:512], pk0[0:HD, :])
                nc.vector.tensor_copy(ktd[0:HD, 512:768], pk1[0:HD, 0:256])
                # gpsimd ring: doesn't queue behind the big wq/wf loads
                nc.gpsimd.dma_start(ktd[HD:128, :], ktd[0:HD, :])

                def v_proj():
                    # Emitted after head 0's logits so the first exp isn't
                    # delayed; PV ops wait on vaug via dataflow deps.
                    for sb in range(NSB):
                        pv = pssm.tile([128, NTB, HD + 1], F32, tag="sm")
                        for kb in range(NKB):
                            nc.tensor.matmul(
                                pv[:, 0, 0:HD],
                                lhsT=xkv_all[:, kb, 128 * sb : 128 * (sb + 1)],
                                rhs=wv_all[:, kb, :],
                                start=(kb == 0),
                                stop=(kb == NKB - 1),
                            )
                        nc.vector.tensor_copy(vaug[sb][:, 0:HD], pv[:, 0, 0:HD])
                        nc.gpsimd.memset(vaug[sb][:, HD : HD + 1], 1.0)

                # ---- Q projection interleaved with attention per block ----
                for mb in range(NKB):
                    pq = psplb.tile([128, 512], F32, tag="plb", name="pq")
                    for kb in range(NKB):
                        nc.tensor.matmul(
                            pq[:],
                            lhsT=wq_all[:, kb, 128 * mb : 128 * (mb + 1)],
                            rhs=xkv_all[:, kb, WIN : WIN + T],
                            start=(kb == 0),
                            stop=(kb == NKB - 1),
                        )
                    nc.scalar.copy(qT_t[mb][:], pq[:])

                    for half in (0, 1):
                        h = 2 * mb + half
                        hb = 64 * half
                        qh = qT_t[mb]
                        plx = [psplb.tile([128, 512], F32, tag="plb", name=f"plx{_i}") for _i in range(3)]
                        probs = awork.tile([128, 3, 512], DT, tag="probs")
                        probsm = awork.tile([128, 3, 512], DT, tag="probsm")
                        for bk in range(3):
                            for bank, off, w, sb, t0 in SEGS:
                                if bank != bk:
                                    continue
                                nc.tensor.matmul(
                                    plx[bk][:, off : off + w],
                                    lhsT=ktd[hb : hb + 64, 128 * sb : 128 * (sb + 1)],
                                    rhs=qh[hb : hb + 64, t0 : t0 + w],
                                    start=True,
                                    stop=True,
                                )
                            nc.scalar.activation(
                                out=probs[:, bk, :],
                                in_=plx[bk][:],
                                func=mybir.ActivationFunctionType.Exp,
                                scale=0.125,
                            )
                            nc.vector.tensor_mul(
                                probsm[:, bk, :], probs[:, bk, :], band_t[:, bk, :]
                            )

                        if h == 0:
                            v_proj()

                        po = pssm.tile([128, NTB, HD + 1], F32, tag="sm")
                        for tb in range(NTB):
                            for k3, (sb, bank, off) in enumerate(PV[tb]):
                                nc.tensor.matmul(
                                    po[:, tb, :],
                                    lhsT=probsm[:, bank, off : off + 128],
                                    rhs=vaug[sb][:],
                                    start=(k3 == 0),
                                    stop=(k3 == 2),
                                )
                        recip = awork.tile([128, NTB, 1], F32, tag="recip")
                        nc.vector.reciprocal(recip[:], po[:, :, HD : HD + 1])
                        for tb in range(NTB):
                            nc.vector.tensor_mul(
                                attn_t[tb][:, 64 * h : 64 * (h + 1)],
                                po[:, tb, 0:HD],
                                recip[:, tb, 0:1].broadcast_to((128, HD)),
                            )

                    # attn -> attnT for this 128-col block via xbar DMA
                    for tb in range(NTB):
                        nc.sync.dma_start_transpose(
                            attnT_t[mb][:, 128 * tb : 128 * (tb + 1)],
                            attn_t[tb][:, 128 * mb : 128 * (mb + 1)],
                        )

                # ---- final projection (bias added on host) ----
                for tb in range(NTB):
                    fo = awork.tile([128, WIDTH], F32, tag="fo")
                    for nh in range(2):
                        pf = pssm.tile([128, 512], F32, tag="sm")
                        for wb in range(NKB):
                            nc.tensor.matmul(
                                pf[:],
                                lhsT=attnT_t[wb][:, 128 * tb : 128 * (tb + 1)],
                                rhs=wf_all[:, wb, 512 * nh : 512 * (nh + 1)],
                                start=(wb == 0),
                                stop=(wb == NKB - 1),
                            )
                        if nh == 0:
                            nc.vector.tensor_copy(fo[:, 0:512], pf[:])
                        else:
                            nc.scalar.copy(fo[:, 512:1024], pf[:])
                    eng = nc.sync if tb % 2 == 0 else nc.scalar
                    eng.dma_start(out_d[128 * tb : 128 * (tb + 1), :], fo[:])

    return nc


def _prep_core_inputs(x, Wq, Wk, Wv, Wf, bf, core):
    bi, ch = divmod(core, 4)
    qs = T * ch
    ks = qs - WIN
    xkvT = np.zeros((WIDTH, KV), np.float32)
    lo = max(ks, 0)
    xkvT[:, lo - ks :] = x[bi, lo : qs + T, :].T

    band = np.zeros((128, 3, 512), np.float32)
    p = np.arange(128)[:, None]
    for bank, off, w, sb, t0 in SEGS:
        f = np.arange(w)[None, :]
        i = t0 + f
        j = 128 * sb + p
        band[:, bank, off : off + w] = (j >= i) & (j <= i + WIN) & (ks + j >= 0)

    return {
        "xkvT": np.ascontiguousarray(xkvT).astype(NPDT),
        "wqT": np.ascontiguousarray(Wq.T).astype(NPDT),
        "wkT": np.ascontiguousarray(Wk.T).astype(NPDT),
        "wvT": np.ascontiguousarray(Wv.T).astype(NPDT),
        "wfT": np.ascontiguousarray(Wf.T).astype(NPDT),
        "band": band.astype(NPDT),
    }


_RUN_KW = {}  # test.py can inject trace=True etc.
_LAST_RESULT = [None]


def kernel(x, segment_pos, Wq, Wk, Wv, Wf, bf):
    x = np.asarray(x, np.float32)
    Wq = np.asarray(Wq, np.float32)
    Wk = np.asarray(Wk, np.float32)
    Wv = np.asarray(Wv, np.float32)
    Wf = np.asarray(Wf, np.float32)
    bf = np.asarray(bf, np.float32)

    nc = build_kernel()
    nc.finalize()
    in_maps = [_prep_core_inputs(x, Wq, Wk, Wv, Wf, bf, c) for c in range(8)]
    res = run_bass_kernel_spmd(nc, in_maps, core_ids=list(range(8)), **_RUN_KW)
    _LAST_RESULT[0] = res

    b, t = x.shape[0], x.shape[1]
    out = np.empty((b, t, WIDTH), np.float32)
    for c in range(8):
        bi, ch = divmod(c, 4)
        out[bi, T * ch : T * (ch + 1)] = res.results[c]["out"] + bf
    return out
